# revision 18
# baseline (speedup 1.0000x reference)
"""Trainium2 Bass kernel for a 2-layer DGL-style GCN (mean aggregation).

Reference computation:
    h_N  = segmean(feat[src] -> dst)                 # [N, 128]
    h    = relu(concat([feat, h_N]) @ W0.T)          # [N, 128]
    h_N2 = segmean(h[src] -> dst)
    out  = concat([h, h_N2]) @ W1.T                  # [N, 64]

Distribution: dst-range sharding over 8 cores (node n owned by core
n // NPC).  Each core aggregates its own nodes exactly (no partial-sum
all-reduce); one AllGather shares z2 = h @ W1b.T between the passes.
Identity used: segmean(h) @ W1b.T == segmean(h @ W1b.T), so the second
gather moves 64-dim rows instead of 128-dim.

Per-core layout: nodes are processed in 25 groups of 512 (4 windows of
128).  For each group the incident edges are bucketed by source-table
chunk (4 chunks of 25600 rows -> int16-addressable for dma_gather) and
padded to a uniform capacity.  One dma_gather per (group, chunk) pulls
all edge source rows into SBUF; a selection matrix
M[slot, v] = (dst_local[slot] == v) * (1/indeg[dst]) is built from an
iota constant with one fused tensor_scalar per 128-slot column; TensorE
accumulates aggT = sum_j G_j.T @ M_j in PSUM, yielding the transposed,
already-mean-scaled aggregation [dims, 512 nodes].  Dense layers run on
the transposed activations; hT stays resident in SBUF between passes.
All 8 cores run one identical program on different data.
"""

import sys

sys.path.insert(0, "/opt/trn_rl_repo")

from contextlib import ExitStack

import numpy as np

import concourse.bass as bass
import concourse.tile as tile
from concourse import bacc, mybir
from concourse.bass_utils import run_bass_kernel_spmd

F32 = mybir.dt.float32
I16 = mybir.dt.int16
P = 128
GRP = 512  # nodes per group (4 windows, one PSUM bank wide)


def _split_sync_waits(nc, max_waits=1):
    """This walrus's codegen rejects instructions carrying more than
    `max_waits` semaphore waits. Hoist the excess onto same-engine nops
    inserted immediately before the offending instruction."""
    import bass_rust

    ctr = 0
    for bb in nc.main_func.blocks:
        insts = bb.instructions
        need = any(
            ins.sync_info is not None and len(ins.sync_info.on_wait) > max_waits
            for ins in insts
        )
        if not need:
            continue
        out = []
        for ins in insts:
            si = ins.sync_info
            if si is not None and len(si.on_wait) > max_waits:
                waits = list(si.on_wait)
                keep, rest = waits[:max_waits], waits[max_waits:]
                while rest:
                    chunk, rest = rest[:max_waits], rest[max_waits:]
                    ctr += 1
                    nop = bass_rust.InstNoOp(
                        name=f"I-waitsplit-{ctr}", engine=ins.engine
                    )
                    nop.sync_info = mybir.SyncInfo(on_wait=chunk, on_update=[])
                    out.append(nop)
                si.on_wait = keep
            out.append(ins)
        insts.clear()
        insts.extend(out)


class _GcnBacc(bacc.Bacc):
    """Bacc whose finalize also splits multi-wait instructions (this
    walrus rejects >1 sync wait on several instruction templates)."""

    def finalize(self):
        if self._finalized:
            return
        self.compile()
        _split_sync_waits(self)
        bass.Bass.finalize(self)


def build_program(cfg):
    """Emit the per-core SPMD program (identical across cores)."""
    N_PAD, D_IN, D_HID, D_OUT = cfg["N_PAD"], cfg["D_IN"], cfg["D_HID"], cfg["D_OUT"]
    C, NPC, KC, NCH = cfg["C"], cfg["NPC"], cfg["KC"], cfg["NCH"]
    CH = N_PAD // NCH  # rows per source-table chunk
    NG = NPC // GRP  # groups per core
    CAP = KC * P  # edge slots per (group, chunk)
    IW = CAP // 16  # idx16 columns per chunk

    nc = _GcnBacc(None)
    feat_t = nc.declare_dram_parameter("feat", [N_PAD, D_IN], F32, isOutput=False)
    featT_t = nc.declare_dram_parameter("featT", [D_IN, NPC], F32, isOutput=False)
    idx_t = nc.declare_dram_parameter("idx16", [NG, P, NCH * IW], I16, isOutput=False)
    dl_t = nc.declare_dram_parameter("dl", [NG, P, NCH * KC], F32, isOutput=False)
    wg_t = nc.declare_dram_parameter("wg", [NG, P, NCH * KC], F32, isOutput=False)
    w0a_t = nc.declare_dram_parameter("w0at", [D_IN, D_HID], F32, isOutput=False)
    w0b_t = nc.declare_dram_parameter("w0bt", [D_IN, D_HID], F32, isOutput=False)
    w1a_t = nc.declare_dram_parameter("w1at", [D_HID, D_OUT], F32, isOutput=False)
    w1b_t = nc.declare_dram_parameter("w1bt", [D_HID, D_OUT], F32, isOutput=False)
    iota_t = nc.declare_dram_parameter("iota", [P, GRP], F32, isOutput=False)
    ident_t = nc.declare_dram_parameter("ident", [D_OUT, D_OUT], F32, isOutput=False)
    outT_t = nc.declare_dram_parameter("outT", [D_OUT, NPC], F32, isOutput=True)

    with ExitStack() as ctx:
        tc = ctx.enter_context(tile.TileContext(nc))

        const = ctx.enter_context(tc.tile_pool(name="const", bufs=1))
        dram = ctx.enter_context(tc.tile_pool(name="dram", bufs=1, space="DRAM"))
        z2_piece = dram.tile([NPC, D_OUT], F32)
        z2_full = dram.tile([C * NPC, D_OUT], F32)

        w0a_s = const.tile([D_IN, D_HID], F32, tag="w0a")
        w0b_s = const.tile([D_IN, D_HID], F32, tag="w0b")
        w1a_s = const.tile([D_HID, D_OUT], F32, tag="w1a")
        w1b_s = const.tile([D_HID, D_OUT], F32, tag="w1b")
        iota_s = const.tile([P, GRP], F32, tag="iota")
        ident_s = const.tile([D_OUT, D_OUT], F32, tag="ident")
        hT_s = const.tile([D_HID, NPC], F32, tag="hT")
        for dst, src in [
            (w0a_s, w0a_t),
            (w0b_s, w0b_t),
            (w1a_s, w1a_t),
            (w1b_s, w1b_t),
            (iota_s, iota_t),
            (ident_s, ident_t),
        ]:
            nc.sync.dma_start(out=dst[:], in_=src[:])

        idxp = ctx.enter_context(tc.tile_pool(name="idxp", bufs=3))
        gp = ctx.enter_context(tc.tile_pool(name="gp", bufs=3))
        mp = ctx.enter_context(tc.tile_pool(name="mp", bufs=4))
        misc = ctx.enter_context(tc.tile_pool(name="misc", bufs=3))
        pa_p = ctx.enter_context(tc.tile_pool(name="pa", bufs=2, space="PSUM"))
        ph_p = ctx.enter_context(tc.tile_pool(name="ph", bufs=2, space="PSUM"))
        pz_p = ctx.enter_context(tc.tile_pool(name="pz", bufs=2, space="PSUM"))
        pzr_p = ctx.enter_context(tc.tile_pool(name="pzr", bufs=2, space="PSUM"))

        def load_group_meta(g, pass_tag):
            idx_s = idxp.tile([P, NCH * IW], I16, tag=f"idx{pass_tag}")
            nc.sync.dma_start(out=idx_s[:], in_=idx_t[g])
            dl_s = idxp.tile([P, NCH * KC], F32, tag=f"dl{pass_tag}")
            nc.sync.dma_start(out=dl_s[:], in_=dl_t[g])
            wg_s = idxp.tile([P, NCH * KC], F32, tag=f"wg{pass_tag}")
            nc.sync.dma_start(out=wg_s[:], in_=wg_t[g])
            return idx_s, dl_s, wg_s

        # SWDGE descriptor ring holds 1024 descriptors; one dma_gather needs
        # ~num_idxs of them, so split each bucket's gather into <=768-index
        # pieces (6 slot-columns of 128).
        GCOLS = 6

        def gather_and_reduce(g, table_ap, elem, psum, gtag, mtag, last_stops):
            """One group's edge gather + selection-matmul accumulation."""
            idx_s, dl_s, wg_s = load_group_meta(g, gtag)
            for c in range(NCH):
                gt = gp.tile([P, KC * elem], F32, tag=gtag)
                for s0 in range(0, KC, GCOLS):
                    s1 = min(s0 + GCOLS, KC)
                    nc.gpsimd.dma_gather(
                        out_ap=gt[:, s0 * elem : s1 * elem].rearrange(
                            "p (k e) -> p k e", e=elem
                        ),
                        in_ap=table_ap[c * CH : (c + 1) * CH, :],
                        idxs_ap=idx_s[:, c * IW + s0 * 8 : c * IW + s1 * 8],
                        num_idxs=(s1 - s0) * P,
                        num_idxs_reg=(s1 - s0) * P,
                        elem_size=elem,
                    )
                for j in range(KC):
                    m = mp.tile([P, GRP], F32, tag=mtag)
                    nc.any.tensor_scalar(
                        out=m[:],
                        in0=iota_s[:],
                        scalar1=dl_s[:, c * KC + j : c * KC + j + 1],
                        scalar2=wg_s[:, c * KC + j : c * KC + j + 1],
                        op0=mybir.AluOpType.is_equal,
                        op1=mybir.AluOpType.mult,
                    )
                    nc.tensor.matmul(
                        psum[:],
                        lhsT=gt[:, j * elem : (j + 1) * elem],
                        rhs=m[:],
                        start=(c == 0 and j == 0),
                        stop=(last_stops and c == NCH - 1 and j == KC - 1),
                    )

        # ---------------- pass 1 ----------------
        for g in range(NG):
            sl = slice(g * GRP, (g + 1) * GRP)
            pa = pa_p.tile([D_IN, GRP], F32, tag="pa")
            gather_and_reduce(g, feat_t, D_IN, pa, "g1", "m1", last_stops=True)

            agg_s = misc.tile([D_IN, GRP], F32, tag="aggs")
            nc.vector.tensor_copy(out=agg_s[:], in_=pa[:])
            fT = misc.tile([D_IN, GRP], F32, tag="fT")
            nc.sync.dma_start(out=fT[:], in_=featT_t[:, sl])

            ph = ph_p.tile([D_HID, GRP], F32, tag="ph")
            nc.tensor.matmul(ph[:], lhsT=w0a_s[:], rhs=fT[:], start=True, stop=False)
            nc.tensor.matmul(ph[:], lhsT=w0b_s[:], rhs=agg_s[:], start=False, stop=True)
            nc.scalar.activation(
                out=hT_s[:, sl], in_=ph[:], func=mybir.ActivationFunctionType.Relu
            )

            pz = pz_p.tile([D_OUT, GRP], F32, tag="pz")
            nc.tensor.matmul(pz[:], lhsT=w1b_s[:], rhs=hT_s[:, sl], start=True, stop=True)
            z2T = misc.tile([D_OUT, GRP], F32, tag="z2T")
            nc.vector.tensor_copy(out=z2T[:], in_=pz[:])
            z2r = misc.tile([P, (GRP // P) * D_OUT], F32, tag="z2r")
            for w4 in range(GRP // P):
                pzr = pzr_p.tile([P, D_OUT], F32, tag="pzr")
                nc.tensor.transpose(
                    out=pzr[:], in_=z2T[:, w4 * P : (w4 + 1) * P], identity=ident_s[:]
                )
                nc.vector.tensor_copy(
                    out=z2r[:, w4 * D_OUT : (w4 + 1) * D_OUT], in_=pzr[:]
                )
            nc.sync.dma_start(
                out=z2_piece[sl, :].rearrange("(w p) e -> p w e", p=P),
                in_=z2r[:].rearrange("p (w e) -> p w e", e=D_OUT),
            )

        # ---------------- all-gather z2 ----------------
        nc.gpsimd.collective_compute(
            "AllGather",
            mybir.AluOpType.bypass,
            replica_groups=[list(range(C))],
            ins=[z2_piece[:]],
            outs=[z2_full[:]],
        )

        # ---------------- pass 2 ----------------
        for g in range(NG):
            sl = slice(g * GRP, (g + 1) * GRP)
            po = pz_p.tile([D_OUT, GRP], F32, tag="pz")
            gather_and_reduce(g, z2_full, D_OUT, po, "g2", "m2", last_stops=False)
            nc.tensor.matmul(
                po[:], lhsT=w1a_s[:], rhs=hT_s[:, sl], start=False, stop=True
            )
            o_s = misc.tile([D_OUT, GRP], F32, tag="os")
            nc.vector.tensor_copy(out=o_s[:], in_=po[:])
            nc.sync.dma_start(out=outT_t[:, sl], in_=o_s[:])

    return nc


def prep_inputs(feat, edge_src, edge_dst, W0, W1, cfg):
    """Host-side index/layout prep. Returns per-core input maps."""
    N, D_IN = feat.shape
    N_PAD, D_HID, D_OUT = cfg["N_PAD"], cfg["D_HID"], cfg["D_OUT"]
    C, NPC, KC, NCH = cfg["C"], cfg["NPC"], cfg["KC"], cfg["NCH"]
    CH = N_PAD // NCH
    NG = NPC // GRP
    CAP = KC * P
    IW = CAP // 16
    E = edge_src.shape[0]

    indeg = np.bincount(edge_dst, minlength=N).astype(np.float32)
    ew = (1.0 / np.maximum(indeg, 1.0))[edge_dst].astype(np.float32)

    grp_of = edge_dst // GRP  # global group id (= core * NG + local group)
    chunk_of = edge_src // CH
    bucket = grp_of * NCH + chunk_of
    n_buckets = C * NG * NCH
    order = np.argsort(bucket, kind="stable")
    src_o = edge_src[order]
    dst_o = edge_dst[order]
    ew_o = ew[order]
    b_o = bucket[order]

    counts = np.bincount(b_o, minlength=n_buckets)
    if counts.max() > CAP:
        raise ValueError(f"bucket overflow: {counts.max()} > {CAP}")
    starts = np.zeros(n_buckets, dtype=np.int64)
    starts[1:] = np.cumsum(counts)[:-1]
    slot = np.arange(E, dtype=np.int64) - starts[b_o]

    idx16 = np.zeros((n_buckets, CAP), dtype=np.int16)
    dl = np.full((n_buckets, CAP), 2.0 * GRP, dtype=np.float32)
    wg = np.zeros((n_buckets, CAP), dtype=np.float32)
    idx16[b_o, slot] = (src_o % CH).astype(np.int16)
    dl[b_o, slot] = (dst_o % GRP).astype(np.float32)
    wg[b_o, slot] = ew_o

    # device layouts --------------------------------------------------
    # idx16: slot i lives at partition i % 16, col i // 16, replicated 8x
    idx16 = idx16.reshape(n_buckets, IW, 16)
    idx_dev = np.tile(idx16.transpose(0, 2, 1), (1, 8, 1))  # [nb, 128, IW]
    # dl/wg: slot i -> partition i % 128, col i // 128
    dl_dev = dl.reshape(n_buckets, KC, P).transpose(0, 2, 1)  # [nb, 128, KC]
    wg_dev = wg.reshape(n_buckets, KC, P).transpose(0, 2, 1)

    idx_dev = idx_dev.reshape(C, NG, NCH, P, IW).transpose(0, 1, 3, 2, 4)
    idx_dev = np.ascontiguousarray(idx_dev).reshape(C, NG, P, NCH * IW)
    dl_dev = dl_dev.reshape(C, NG, NCH, P, KC).transpose(0, 1, 3, 2, 4)
    dl_dev = np.ascontiguousarray(dl_dev).reshape(C, NG, P, NCH * KC)
    wg_dev = wg_dev.reshape(C, NG, NCH, P, KC).transpose(0, 1, 3, 2, 4)
    wg_dev = np.ascontiguousarray(wg_dev).reshape(C, NG, P, NCH * KC)

    feat_pad = np.zeros((N_PAD, D_IN), dtype=np.float32)
    feat_pad[:N] = feat
    featT = np.zeros((D_IN, C * NPC), dtype=np.float32)
    featT[:, :N] = feat.T

    w0a = np.ascontiguousarray(W0[:, :D_IN].T)
    w0b = np.ascontiguousarray(W0[:, D_IN:].T)
    w1a = np.ascontiguousarray(W1[:, :D_HID].T)
    w1b = np.ascontiguousarray(W1[:, D_HID:].T)
    iota = np.tile(np.arange(GRP, dtype=np.float32), (P, 1))
    ident = np.eye(D_OUT, dtype=np.float32)

    in_maps = []
    for c in range(C):
        in_maps.append(
            {
                "feat": feat_pad,
                "featT": np.ascontiguousarray(featT[:, c * NPC : (c + 1) * NPC]),
                "idx16": idx_dev[c],
                "dl": dl_dev[c],
                "wg": wg_dev[c],
                "w0at": w0a,
                "w0bt": w0b,
                "w1at": w1a,
                "w1bt": w1b,
                "iota": iota,
                "ident": ident,
            }
        )
    return in_maps


_PROGRAM_CACHE = {}


def make_cfg(N, E, D_IN, D_HID, D_OUT, C=8):
    NPC = -(-N // (C * GRP)) * GRP  # per-core nodes, multiple of 512
    N_PAD = C * NPC
    NCH = 4
    assert N_PAD % NCH == 0 and N_PAD // NCH <= 32768
    # capacity per (group, chunk) bucket: mean + 5 sigma, rounded to 128
    mean_b = E / (N / GRP) / NCH
    cap = mean_b + 5.0 * np.sqrt(mean_b) + 2
    KC = max(1, int(np.ceil(cap / P)))
    return {
        "N": N,
        "N_PAD": N_PAD,
        "D_IN": D_IN,
        "D_HID": D_HID,
        "D_OUT": D_OUT,
        "C": C,
        "NPC": NPC,
        "KC": KC,
        "NCH": NCH,
    }


def _run(feat, edge_src, edge_dst, W0, W1, C=8, trace=False):
    N, D_IN = feat.shape
    cfg = make_cfg(N, edge_src.shape[0], D_IN, W0.shape[0], W1.shape[0], C)
    # bump capacity if the actual edge distribution overflows the estimate
    for _ in range(8):
        try:
            in_maps = prep_inputs(feat, edge_src, edge_dst, W0, W1, cfg)
            break
        except ValueError:
            cfg["KC"] += 1
    key = tuple(sorted(cfg.items()))
    if key not in _PROGRAM_CACHE:
        nc_new = build_program(cfg)
        nc_new.finalize()
        _PROGRAM_CACHE[key] = nc_new
    nc = _PROGRAM_CACHE[key]

    res = run_bass_kernel_spmd(nc, in_maps, core_ids=list(range(C)), trace=trace)
    pieces = [res.results[c]["outT"].T for c in range(C)]  # [NPC, D_OUT]
    out = np.concatenate(pieces, axis=0)[:N]
    return np.ascontiguousarray(out), res


def bench(feat, edge_src, edge_dst, W0, W1, C=8, iters=10):
    """Time device execution of the compiled program: inputs pre-staged on
    device, jit without donation, min over `iters` calls."""
    import time

    import jax
    from jax.sharding import Mesh, NamedSharding, PartitionSpec

    try:
        from jax.experimental.shard_map import shard_map
    except ImportError:
        from jax.shard_map import shard_map
    from concourse import bass2jax
    from concourse.bass2jax import _bass_exec_p

    feat = np.asarray(feat, dtype=np.float32)
    edge_src = np.asarray(edge_src, dtype=np.int32)
    edge_dst = np.asarray(edge_dst, dtype=np.int32)
    W0 = np.asarray(W0, dtype=np.float32)
    W1 = np.asarray(W1, dtype=np.float32)
    cfg = make_cfg(feat.shape[0], edge_src.shape[0], feat.shape[1], W0.shape[0], W1.shape[0], C)
    for _ in range(8):
        try:
            in_maps = prep_inputs(feat, edge_src, edge_dst, W0, W1, cfg)
            break
        except ValueError:
            cfg["KC"] += 1
    key = tuple(sorted(cfg.items()))
    if key not in _PROGRAM_CACHE:
        nc_new = build_program(cfg)
        nc_new.finalize()
        _PROGRAM_CACHE[key] = nc_new
    nc = _PROGRAM_CACHE[key]

    bass2jax.install_neuronx_cc_hook()
    import concourse.mybir as mb

    part_name = nc.partition_id_tensor.name if nc.partition_id_tensor else None
    in_names, out_names, out_avals, zero_outs = [], [], [], []
    for alloc in nc.m.functions[0].allocations:
        if not isinstance(alloc, mb.MemoryLocationSet):
            continue
        name = alloc.memorylocations[0].name
        if alloc.kind == "ExternalInput":
            if name != part_name:
                in_names.append(name)
        elif alloc.kind == "ExternalOutput":
            shape = tuple(alloc.tensor_shape)
            dtype = mb.dt.np(alloc.dtype)
            out_names.append(name)
            out_avals.append(jax.core.ShapedArray(shape, dtype))
            zero_outs.append(np.zeros(shape, dtype))
    n_params = len(in_names)
    all_in_names = in_names + out_names
    if part_name is not None:
        all_in_names.append(part_name)

    def _body(*args):
        operands = list(args)
        if part_name is not None:
            operands.append(bass2jax.partition_id_tensor())
        return tuple(
            _bass_exec_p.bind(
                *operands,
                out_avals=tuple(out_avals),
                in_names=tuple(all_in_names),
                out_names=tuple(out_names),
                lowering_input_output_aliases=(),
                sim_require_finite=True,
                sim_require_nnan=True,
                nc=nc,
            )
        )

    devices = jax.devices()[:C]
    mesh = Mesh(np.asarray(devices), ("core",))
    spec = PartitionSpec("core")
    n_args = n_params + len(out_names)
    fn = jax.jit(
        shard_map(
            _body,
            mesh=mesh,
            in_specs=(spec,) * n_args,
            out_specs=(spec,) * len(out_names),
            check_rep=False,
        )
    )
    concat_in = [
        np.concatenate([np.asarray(in_maps[c][nm]) for c in range(C)], axis=0)
        for nm in in_names
    ] + [np.zeros((C * z.shape[0], *z.shape[1:]), z.dtype) for z in zero_outs]
    sharding = NamedSharding(mesh, spec)
    dev_in = [jax.device_put(a, sharding) for a in concat_in]
    # warmup (compiles + first exec)
    r = fn(*dev_in)
    jax.block_until_ready(r)
    times = []
    for _ in range(iters):
        t0 = time.perf_counter()
        r = fn(*dev_in)
        jax.block_until_ready(r)
        times.append(time.perf_counter() - t0)
    return {
        "min_s": min(times),
        "median_s": sorted(times)[len(times) // 2],
        "all_s": times,
        "out": np.asarray(r[0]),
        "out_names": out_names,
        "cfg": cfg,
    }


def kernel(feat, edge_src, edge_dst, W0, W1):
    out, _ = _run(
        np.asarray(feat, dtype=np.float32),
        np.asarray(edge_src, dtype=np.int32),
        np.asarray(edge_dst, dtype=np.int32),
        np.asarray(W0, dtype=np.float32),
        np.asarray(W1, dtype=np.float32),
    )
    return out


# revision 24
# speedup vs baseline: 103.6418x; 103.6418x over previous
"""Trainium2 Bass kernel for a 2-layer DGL-style GCN (mean aggregation).

Reference computation:
    h_N  = segmean(feat[src] -> dst)                 # [N, 128]
    h    = relu(concat([feat, h_N]) @ W0.T)          # [N, 128]
    h_N2 = segmean(h[src] -> dst)
    out  = concat([h, h_N2]) @ W1.T                  # [N, 64]

Distribution: dst-range sharding over 8 cores (node n owned by core
n // NPC).  Each core aggregates its own nodes exactly (no partial-sum
all-reduce); one AllGather shares z2 = h @ W1b.T between the passes.
Identity used: segmean(h) @ W1b.T == segmean(h @ W1b.T), so the second
gather moves 64-dim rows instead of 128-dim.

Per-core layout: nodes are processed in 25 groups of 512 (4 windows of
128).  For each group the incident edges are bucketed by source-table
chunk (4 chunks of 25600 rows -> int16-addressable for dma_gather) and
padded to a uniform capacity.  One dma_gather per (group, chunk) pulls
all edge source rows into SBUF; a selection matrix
M[slot, v] = (dst_local[slot] == v) * (1/indeg[dst]) is built from an
iota constant with one fused tensor_scalar per 128-slot column; TensorE
accumulates aggT = sum_j G_j.T @ M_j in PSUM, yielding the transposed,
already-mean-scaled aggregation [dims, 512 nodes].  Dense layers run on
the transposed activations; hT stays resident in SBUF between passes.
All 8 cores run one identical program on different data.
"""

import sys

sys.path.insert(0, "/opt/trn_rl_repo")

from contextlib import ExitStack

import numpy as np

import concourse.bass as bass
import concourse.tile as tile
from concourse import bacc, mybir
from concourse.bass_utils import run_bass_kernel_spmd

F32 = mybir.dt.float32
I16 = mybir.dt.int16
P = 128
GRP = 512  # nodes per group (4 windows, one PSUM bank wide)


def _split_sync_waits(nc, max_waits=1):
    """This walrus's codegen rejects instructions carrying more than
    `max_waits` semaphore waits. Hoist the excess onto same-engine nops
    inserted immediately before the offending instruction."""
    import bass_rust

    ctr = 0
    for bb in nc.main_func.blocks:
        insts = bb.instructions
        need = any(
            ins.sync_info is not None and len(ins.sync_info.on_wait) > max_waits
            for ins in insts
        )
        if not need:
            continue
        out = []
        for ins in insts:
            si = ins.sync_info
            if si is not None and len(si.on_wait) > max_waits:
                waits = list(si.on_wait)
                keep, rest = waits[:max_waits], waits[max_waits:]
                while rest:
                    chunk, rest = rest[:max_waits], rest[max_waits:]
                    ctr += 1
                    nop = bass_rust.InstNoOp(
                        name=f"I-waitsplit-{ctr}", engine=ins.engine
                    )
                    nop.sync_info = mybir.SyncInfo(on_wait=chunk, on_update=[])
                    out.append(nop)
                si.on_wait = keep
            out.append(ins)
        insts.clear()
        insts.extend(out)


class _GcnBacc(bacc.Bacc):
    """Bacc whose finalize also splits multi-wait instructions (this
    walrus rejects >1 sync wait on several instruction templates)."""

    def finalize(self):
        if self._finalized:
            return
        self.compile()
        _split_sync_waits(self)
        bass.Bass.finalize(self)


def build_program(cfg, reps=1):
    """Emit the per-core SPMD program (identical across cores).

    reps>1 repeats the whole computation (for timing via slope: the axon
    dispatch floor is ~80ms, far above the kernel itself)."""
    N_PAD, D_IN, D_HID, D_OUT = cfg["N_PAD"], cfg["D_IN"], cfg["D_HID"], cfg["D_OUT"]
    C, NPC, KC, NCH = cfg["C"], cfg["NPC"], cfg["KC"], cfg["NCH"]
    CH = N_PAD // NCH  # rows per source-table chunk
    NG = NPC // GRP  # groups per core
    CAP = KC * P  # edge slots per (group, chunk)
    IW = CAP // 16  # idx16 columns per chunk

    nc = _GcnBacc(None)
    feat_t = nc.declare_dram_parameter("feat", [N_PAD, D_IN], F32, isOutput=False)
    featT_t = nc.declare_dram_parameter("featT", [D_IN, NPC], F32, isOutput=False)
    idx_t = nc.declare_dram_parameter("idx16", [NG, P, NCH * IW], I16, isOutput=False)
    dl_t = nc.declare_dram_parameter("dl", [NG, P, NCH * KC], F32, isOutput=False)
    wg_t = nc.declare_dram_parameter("wg", [NG, P, NCH * KC], F32, isOutput=False)
    w0a_t = nc.declare_dram_parameter("w0at", [D_IN, D_HID], F32, isOutput=False)
    w0b_t = nc.declare_dram_parameter("w0bt", [D_IN, D_HID], F32, isOutput=False)
    w1a_t = nc.declare_dram_parameter("w1at", [D_HID, D_OUT], F32, isOutput=False)
    w1b_t = nc.declare_dram_parameter("w1bt", [D_HID, D_OUT], F32, isOutput=False)
    iota_t = nc.declare_dram_parameter("iota", [P, GRP], F32, isOutput=False)
    ident_t = nc.declare_dram_parameter("ident", [D_OUT, D_OUT], F32, isOutput=False)
    outT_t = nc.declare_dram_parameter("outT", [D_OUT, NPC], F32, isOutput=True)

    with ExitStack() as ctx:
        tc = ctx.enter_context(tile.TileContext(nc))

        const = ctx.enter_context(tc.tile_pool(name="const", bufs=1))
        dram = ctx.enter_context(tc.tile_pool(name="dram", bufs=1, space="DRAM"))
        z2_piece = dram.tile([NPC, D_OUT], F32)
        z2_full = dram.tile([C * NPC, D_OUT], F32)

        w0a_s = const.tile([D_IN, D_HID], F32, tag="w0a")
        w0b_s = const.tile([D_IN, D_HID], F32, tag="w0b")
        w1a_s = const.tile([D_HID, D_OUT], F32, tag="w1a")
        w1b_s = const.tile([D_HID, D_OUT], F32, tag="w1b")
        iota_s = const.tile([P, GRP], F32, tag="iota")
        ident_s = const.tile([D_OUT, D_OUT], F32, tag="ident")
        hT_s = const.tile([D_HID, NPC], F32, tag="hT")
        for dst, src in [
            (w0a_s, w0a_t),
            (w0b_s, w0b_t),
            (w1a_s, w1a_t),
            (w1b_s, w1b_t),
            (iota_s, iota_t),
            (ident_s, ident_t),
        ]:
            nc.sync.dma_start(out=dst[:], in_=src[:])

        idxp = ctx.enter_context(tc.tile_pool(name="idxp", bufs=3))
        gp = ctx.enter_context(tc.tile_pool(name="gp", bufs=3))
        mp = ctx.enter_context(tc.tile_pool(name="mp", bufs=4))
        misc = ctx.enter_context(tc.tile_pool(name="misc", bufs=3))
        pa_p = ctx.enter_context(tc.tile_pool(name="pa", bufs=2, space="PSUM"))
        ph_p = ctx.enter_context(tc.tile_pool(name="ph", bufs=2, space="PSUM"))
        pz_p = ctx.enter_context(tc.tile_pool(name="pz", bufs=2, space="PSUM"))
        pzr_p = ctx.enter_context(tc.tile_pool(name="pzr", bufs=2, space="PSUM"))

        def load_group_meta(g, pass_tag):
            idx_s = idxp.tile([P, NCH * IW], I16, tag=f"idx{pass_tag}")
            nc.sync.dma_start(out=idx_s[:], in_=idx_t[g])
            dl_s = idxp.tile([P, NCH * KC], F32, tag=f"dl{pass_tag}")
            nc.sync.dma_start(out=dl_s[:], in_=dl_t[g])
            wg_s = idxp.tile([P, NCH * KC], F32, tag=f"wg{pass_tag}")
            nc.sync.dma_start(out=wg_s[:], in_=wg_t[g])
            return idx_s, dl_s, wg_s

        # SWDGE descriptor ring holds 1024 descriptors; one dma_gather needs
        # ~num_idxs of them, so split each bucket's gather into <=768-index
        # pieces (6 slot-columns of 128).
        GCOLS = 6

        def gather_and_reduce(g, table_ap, elem, psum, gtag, mtag, last_stops):
            """One group's edge gather + selection-matmul accumulation."""
            idx_s, dl_s, wg_s = load_group_meta(g, gtag)
            for c in range(NCH):
                gt = gp.tile([P, KC * elem], F32, tag=gtag)
                for s0 in range(0, KC, GCOLS):
                    s1 = min(s0 + GCOLS, KC)
                    nc.gpsimd.dma_gather(
                        out_ap=gt[:, s0 * elem : s1 * elem].rearrange(
                            "p (k e) -> p k e", e=elem
                        ),
                        in_ap=table_ap[c * CH : (c + 1) * CH, :],
                        idxs_ap=idx_s[:, c * IW + s0 * 8 : c * IW + s1 * 8],
                        num_idxs=(s1 - s0) * P,
                        num_idxs_reg=(s1 - s0) * P,
                        elem_size=elem,
                    )
                for j in range(KC):
                    m = mp.tile([P, GRP], F32, tag=mtag)
                    nc.any.tensor_scalar(
                        out=m[:],
                        in0=iota_s[:],
                        scalar1=dl_s[:, c * KC + j : c * KC + j + 1],
                        scalar2=wg_s[:, c * KC + j : c * KC + j + 1],
                        op0=mybir.AluOpType.is_equal,
                        op1=mybir.AluOpType.mult,
                    )
                    nc.tensor.matmul(
                        psum[:],
                        lhsT=gt[:, j * elem : (j + 1) * elem],
                        rhs=m[:],
                        start=(c == 0 and j == 0),
                        stop=(last_stops and c == NCH - 1 and j == KC - 1),
                    )

        # ---------------- pass 1 ----------------
        for _rep in range(reps):
            for g in range(NG):
                sl = slice(g * GRP, (g + 1) * GRP)
                pa = pa_p.tile([D_IN, GRP], F32, tag="pa")
                gather_and_reduce(g, feat_t, D_IN, pa, "g1", "m1", last_stops=True)

                agg_s = misc.tile([D_IN, GRP], F32, tag="aggs")
                nc.vector.tensor_copy(out=agg_s[:], in_=pa[:])
                fT = misc.tile([D_IN, GRP], F32, tag="fT")
                nc.sync.dma_start(out=fT[:], in_=featT_t[:, sl])

                ph = ph_p.tile([D_HID, GRP], F32, tag="ph")
                nc.tensor.matmul(ph[:], lhsT=w0a_s[:], rhs=fT[:], start=True, stop=False)
                nc.tensor.matmul(ph[:], lhsT=w0b_s[:], rhs=agg_s[:], start=False, stop=True)
                nc.scalar.activation(
                    out=hT_s[:, sl], in_=ph[:], func=mybir.ActivationFunctionType.Relu
                )

                pz = pz_p.tile([D_OUT, GRP], F32, tag="pz")
                nc.tensor.matmul(pz[:], lhsT=w1b_s[:], rhs=hT_s[:, sl], start=True, stop=True)
                z2T = misc.tile([D_OUT, GRP], F32, tag="z2T")
                nc.vector.tensor_copy(out=z2T[:], in_=pz[:])
                z2r = misc.tile([P, (GRP // P) * D_OUT], F32, tag="z2r")
                for w4 in range(GRP // P):
                    pzr = pzr_p.tile([P, D_OUT], F32, tag="pzr")
                    nc.tensor.transpose(
                        out=pzr[:], in_=z2T[:, w4 * P : (w4 + 1) * P], identity=ident_s[:]
                    )
                    nc.vector.tensor_copy(
                        out=z2r[:, w4 * D_OUT : (w4 + 1) * D_OUT], in_=pzr[:]
                    )
                nc.sync.dma_start(
                    out=z2_piece[sl, :].rearrange("(w p) e -> p w e", p=P),
                    in_=z2r[:].rearrange("p (w e) -> p w e", e=D_OUT),
                )

            # ---------------- all-gather z2 ----------------
            nc.gpsimd.collective_compute(
                "AllGather",
                mybir.AluOpType.bypass,
                replica_groups=[list(range(C))],
                ins=[z2_piece[:]],
                outs=[z2_full[:]],
            )

            # ---------------- pass 2 ----------------
            for g in range(NG):
                sl = slice(g * GRP, (g + 1) * GRP)
                po = pz_p.tile([D_OUT, GRP], F32, tag="pz")
                gather_and_reduce(g, z2_full, D_OUT, po, "g2", "m2", last_stops=False)
                nc.tensor.matmul(
                    po[:], lhsT=w1a_s[:], rhs=hT_s[:, sl], start=False, stop=True
                )
                o_s = misc.tile([D_OUT, GRP], F32, tag="os")
                nc.vector.tensor_copy(out=o_s[:], in_=po[:])
                nc.sync.dma_start(out=outT_t[:, sl], in_=o_s[:])

    return nc


def prep_inputs(feat, edge_src, edge_dst, W0, W1, cfg):
    """Host-side index/layout prep. Returns per-core input maps."""
    N, D_IN = feat.shape
    N_PAD, D_HID, D_OUT = cfg["N_PAD"], cfg["D_HID"], cfg["D_OUT"]
    C, NPC, KC, NCH = cfg["C"], cfg["NPC"], cfg["KC"], cfg["NCH"]
    CH = N_PAD // NCH
    NG = NPC // GRP
    CAP = KC * P
    IW = CAP // 16
    E = edge_src.shape[0]

    indeg = np.bincount(edge_dst, minlength=N).astype(np.float32)
    ew = (1.0 / np.maximum(indeg, 1.0))[edge_dst].astype(np.float32)

    grp_of = edge_dst // GRP  # global group id (= core * NG + local group)
    chunk_of = edge_src // CH
    bucket = grp_of * NCH + chunk_of
    n_buckets = C * NG * NCH
    order = np.argsort(bucket, kind="stable")
    src_o = edge_src[order]
    dst_o = edge_dst[order]
    ew_o = ew[order]
    b_o = bucket[order]

    counts = np.bincount(b_o, minlength=n_buckets)
    if counts.max() > CAP:
        raise ValueError(f"bucket overflow: {counts.max()} > {CAP}")
    starts = np.zeros(n_buckets, dtype=np.int64)
    starts[1:] = np.cumsum(counts)[:-1]
    slot = np.arange(E, dtype=np.int64) - starts[b_o]

    idx16 = np.zeros((n_buckets, CAP), dtype=np.int16)
    dl = np.full((n_buckets, CAP), 2.0 * GRP, dtype=np.float32)
    wg = np.zeros((n_buckets, CAP), dtype=np.float32)
    idx16[b_o, slot] = (src_o % CH).astype(np.int16)
    dl[b_o, slot] = (dst_o % GRP).astype(np.float32)
    wg[b_o, slot] = ew_o

    # device layouts --------------------------------------------------
    # idx16: slot i lives at partition i % 16, col i // 16, replicated 8x
    idx16 = idx16.reshape(n_buckets, IW, 16)
    idx_dev = np.tile(idx16.transpose(0, 2, 1), (1, 8, 1))  # [nb, 128, IW]
    # dl/wg: slot i -> partition i % 128, col i // 128
    dl_dev = dl.reshape(n_buckets, KC, P).transpose(0, 2, 1)  # [nb, 128, KC]
    wg_dev = wg.reshape(n_buckets, KC, P).transpose(0, 2, 1)

    idx_dev = idx_dev.reshape(C, NG, NCH, P, IW).transpose(0, 1, 3, 2, 4)
    idx_dev = np.ascontiguousarray(idx_dev).reshape(C, NG, P, NCH * IW)
    dl_dev = dl_dev.reshape(C, NG, NCH, P, KC).transpose(0, 1, 3, 2, 4)
    dl_dev = np.ascontiguousarray(dl_dev).reshape(C, NG, P, NCH * KC)
    wg_dev = wg_dev.reshape(C, NG, NCH, P, KC).transpose(0, 1, 3, 2, 4)
    wg_dev = np.ascontiguousarray(wg_dev).reshape(C, NG, P, NCH * KC)

    feat_pad = np.zeros((N_PAD, D_IN), dtype=np.float32)
    feat_pad[:N] = feat
    featT = np.zeros((D_IN, C * NPC), dtype=np.float32)
    featT[:, :N] = feat.T

    w0a = np.ascontiguousarray(W0[:, :D_IN].T)
    w0b = np.ascontiguousarray(W0[:, D_IN:].T)
    w1a = np.ascontiguousarray(W1[:, :D_HID].T)
    w1b = np.ascontiguousarray(W1[:, D_HID:].T)
    iota = np.tile(np.arange(GRP, dtype=np.float32), (P, 1))
    ident = np.eye(D_OUT, dtype=np.float32)

    in_maps = []
    for c in range(C):
        in_maps.append(
            {
                "feat": feat_pad,
                "featT": np.ascontiguousarray(featT[:, c * NPC : (c + 1) * NPC]),
                "idx16": idx_dev[c],
                "dl": dl_dev[c],
                "wg": wg_dev[c],
                "w0at": w0a,
                "w0bt": w0b,
                "w1at": w1a,
                "w1bt": w1b,
                "iota": iota,
                "ident": ident,
            }
        )
    return in_maps


_PROGRAM_CACHE = {}


def make_cfg(N, E, D_IN, D_HID, D_OUT, C=8):
    NPC = -(-N // (C * GRP)) * GRP  # per-core nodes, multiple of 512
    N_PAD = C * NPC
    NCH = 4
    assert N_PAD % NCH == 0 and N_PAD // NCH <= 32768
    # capacity per (group, chunk) bucket: mean + 5 sigma, rounded to 128
    mean_b = E / (N / GRP) / NCH
    cap = mean_b + 5.0 * np.sqrt(mean_b) + 2
    KC = max(1, int(np.ceil(cap / P)))
    return {
        "N": N,
        "N_PAD": N_PAD,
        "D_IN": D_IN,
        "D_HID": D_HID,
        "D_OUT": D_OUT,
        "C": C,
        "NPC": NPC,
        "KC": KC,
        "NCH": NCH,
    }


def _run(feat, edge_src, edge_dst, W0, W1, C=8, trace=False):
    N, D_IN = feat.shape
    cfg = make_cfg(N, edge_src.shape[0], D_IN, W0.shape[0], W1.shape[0], C)
    # bump capacity if the actual edge distribution overflows the estimate
    for _ in range(8):
        try:
            in_maps = prep_inputs(feat, edge_src, edge_dst, W0, W1, cfg)
            break
        except ValueError:
            cfg["KC"] += 1
    key = (tuple(sorted(cfg.items())), 1)
    if key not in _PROGRAM_CACHE:
        nc_new = build_program(cfg)
        nc_new.finalize()
        _PROGRAM_CACHE[key] = nc_new
    nc = _PROGRAM_CACHE[key]

    res = run_bass_kernel_spmd(nc, in_maps, core_ids=list(range(C)), trace=trace)
    pieces = [res.results[c]["outT"].T for c in range(C)]  # [NPC, D_OUT]
    out = np.concatenate(pieces, axis=0)[:N]
    return np.ascontiguousarray(out), res


def bench(feat, edge_src, edge_dst, W0, W1, C=8, iters=10, reps=1):
    """Time device execution of the compiled program: inputs pre-staged on
    device, jit without donation, min over `iters` calls."""
    import time

    import jax
    from jax.sharding import Mesh, NamedSharding, PartitionSpec

    try:
        from jax.experimental.shard_map import shard_map
    except ImportError:
        from jax.shard_map import shard_map
    from concourse import bass2jax
    from concourse.bass2jax import _bass_exec_p

    feat = np.asarray(feat, dtype=np.float32)
    edge_src = np.asarray(edge_src, dtype=np.int32)
    edge_dst = np.asarray(edge_dst, dtype=np.int32)
    W0 = np.asarray(W0, dtype=np.float32)
    W1 = np.asarray(W1, dtype=np.float32)
    cfg = make_cfg(feat.shape[0], edge_src.shape[0], feat.shape[1], W0.shape[0], W1.shape[0], C)
    for _ in range(8):
        try:
            in_maps = prep_inputs(feat, edge_src, edge_dst, W0, W1, cfg)
            break
        except ValueError:
            cfg["KC"] += 1
    key = (tuple(sorted(cfg.items())), reps)
    if key not in _PROGRAM_CACHE:
        nc_new = build_program(cfg, reps=reps)
        nc_new.finalize()
        _PROGRAM_CACHE[key] = nc_new
    nc = _PROGRAM_CACHE[key]

    bass2jax.install_neuronx_cc_hook()
    import concourse.mybir as mb

    part_name = nc.partition_id_tensor.name if nc.partition_id_tensor else None
    in_names, out_names, out_avals, zero_outs = [], [], [], []
    for alloc in nc.m.functions[0].allocations:
        if not isinstance(alloc, mb.MemoryLocationSet):
            continue
        name = alloc.memorylocations[0].name
        if alloc.kind == "ExternalInput":
            if name != part_name:
                in_names.append(name)
        elif alloc.kind == "ExternalOutput":
            shape = tuple(alloc.tensor_shape)
            dtype = mb.dt.np(alloc.dtype)
            out_names.append(name)
            out_avals.append(jax.core.ShapedArray(shape, dtype))
            zero_outs.append(np.zeros(shape, dtype))
    n_params = len(in_names)
    all_in_names = in_names + out_names
    if part_name is not None:
        all_in_names.append(part_name)

    def _body(*args):
        operands = list(args)
        if part_name is not None:
            operands.append(bass2jax.partition_id_tensor())
        return tuple(
            _bass_exec_p.bind(
                *operands,
                out_avals=tuple(out_avals),
                in_names=tuple(all_in_names),
                out_names=tuple(out_names),
                lowering_input_output_aliases=(),
                sim_require_finite=True,
                sim_require_nnan=True,
                nc=nc,
            )
        )

    devices = jax.devices()[:C]
    mesh = Mesh(np.asarray(devices), ("core",))
    spec = PartitionSpec("core")
    n_args = n_params + len(out_names)
    fn = jax.jit(
        shard_map(
            _body,
            mesh=mesh,
            in_specs=(spec,) * n_args,
            out_specs=(spec,) * len(out_names),
            check_rep=False,
        )
    )
    concat_in = [
        np.concatenate([np.asarray(in_maps[c][nm]) for c in range(C)], axis=0)
        for nm in in_names
    ] + [np.zeros((C * z.shape[0], *z.shape[1:]), z.dtype) for z in zero_outs]
    sharding = NamedSharding(mesh, spec)
    dev_in = [jax.device_put(a, sharding) for a in concat_in]
    # warmup (compiles + first exec)
    r = fn(*dev_in)
    jax.block_until_ready(r)
    times = []
    for _ in range(iters):
        t0 = time.perf_counter()
        r = fn(*dev_in)
        jax.block_until_ready(r)
        times.append(time.perf_counter() - t0)
    return {
        "min_s": min(times),
        "median_s": sorted(times)[len(times) // 2],
        "all_s": times,
        "out": np.asarray(r[0]),
        "out_names": out_names,
        "cfg": cfg,
    }


def kernel(feat, edge_src, edge_dst, W0, W1):
    out, _ = _run(
        np.asarray(feat, dtype=np.float32),
        np.asarray(edge_src, dtype=np.int32),
        np.asarray(edge_dst, dtype=np.int32),
        np.asarray(W0, dtype=np.float32),
        np.asarray(W1, dtype=np.float32),
    )
    return out
